# revision 24
# baseline (speedup 1.0000x reference)
"""ContactAwareLoss Trainium2 kernel.

Strategy: pure data-parallel over batch (512 rows -> 8 cores x 64 rows).
Each core computes four partial sums over its shard:
  [0] sum_{t,h} probs2 * |dist - 0.1|            (contact distance, unnormalized)
  [1] sum_{j,h} probs2[j+1] * ||r[j+1]-r[j]||     (contact velocity, unnormalized)
  [2] sum_{t,h} first_contact * (5-tap sum of |second diff of dist|)
  [3] sum first_contact                           (count)
The host divides by the global element counts / count and applies the ramp.

On-chip layout: partition p = half*64 + b  (sequence halved so 64 batch rows
fill 128 partitions); free dim = time within the half, processed in W-wide
chunks with a 3-element halo on both sides.  The halo at the half boundary is
filled with real neighbour data via small extra DMAs; the halo at the global
sequence ends is zero-filled and the affected contributions are masked by
zeroing q/vd edge columns (smoothness valid t in [3, seq-3), velocity valid
j in [0, seq-1)).

Engine split (final):
 - DMA: all big input loads ride SWDGE (nc.gpsimd.dma_start).  Each SWDGE
   dma_start binds to ONE SDMA engine pair (round-robin over 8 pairs), so a
   chunk's inputs are issued as 8 equal ~0.4MB instructions (hand 4, obj 2,
   probs 2) putting one stream on every pair; the HWDGE rings (sync/scalar)
   pin all >4KB descriptors to engines 64/65 only (~52 GB/s) and carry just
   the tiny probs halo.  hand/obj are cast fp32->bf16 in flight (SWDGE-only
   feature): halves SBUF writes and makes the r subtraction a 2x bf16 op.
 - GpSimd: SWDGE descriptor emission only (no compute -- gpsimd tensor ops
   measured ~8 G elem/s, 15x slower than DVE).
 - TensorE: the stride-3 coordinate sums (d2, v2) as identity-matmul
   accumulations into PSUM (<=512-col groups per bank); ScalarE Sqrt reads
   PSUM directly.
 - DVE: diffs/movsum in bf16 2x mode, first-contact mask fused to two ops
   (is_gt + is_le/logical_and STT with count accum), weighted-sum dots.
 - ScalarE: Square / Sqrt / Abs (contiguous APs; sqrt-d ordered ahead of
   dsq so the smoothness chain unblocks early).
"""

import numpy as np

BS, SEQ = 512, 4096
N_CORES = 8
W_FULL = 512  # chunk width (per half-sequence)
R_BROADCAST = True  # single hand-obj sub via zero-stride obj AP


def build_nc(bs_local, seq, W):
    import concourse.bass as bass
    import concourse.bacc as bacc
    import concourse.tile as tile
    from concourse import mybir

    f32 = mybir.dt.float32
    bf16 = mybir.dt.bfloat16
    Alu = mybir.AluOpType
    Act = mybir.ActivationFunctionType

    P = 2 * bs_local          # partitions used
    HS = seq // 2             # timesteps per partition row
    assert HS % W == 0
    C = HS // W               # chunks
    E = W + 6                 # chunk width incl. +-3 halo
    H = P // 2

    nc = bacc.Bacc("TRN2", target_bir_lowering=False, debug=False)
    hand = nc.dram_tensor("pred_hand_pos", [bs_local, seq, 2, 3], f32, kind="ExternalInput")
    obj = nc.dram_tensor("pred_obj_pos", [bs_local, seq, 3], f32, kind="ExternalInput")
    probs = nc.dram_tensor("contact_probs", [bs_local, seq, 3], f32, kind="ExternalInput")
    partials = nc.dram_tensor("partials", [P, 4], f32, kind="ExternalOutput")

    def dram_ap(t, offset, dims):
        return bass.AP(tensor=t, offset=offset, ap=[list(d) for d in dims])

    with tile.TileContext(nc) as tc:
        import contextlib
        with contextlib.ExitStack() as ctx:
            inp = ctx.enter_context(tc.tile_pool(name="inp", bufs=3))
            work = ctx.enter_context(tc.tile_pool(name="work", bufs=2))
            singles = ctx.enter_context(tc.tile_pool(name="singles", bufs=1))
            psum = ctx.enter_context(tc.psum_pool(name="ps", bufs=1))

            l1s = singles.tile([P, C], f32)
            l2s = singles.tile([P, C], f32)
            sms = singles.tile([P, C], f32)
            cns = singles.tile([P, C], f32)
            outt = singles.tile([P, 4], f32)
            c_neg01 = singles.tile([P, 1], f32)
            nc.vector.memset(c_neg01[:], -0.1)
            c_neg05 = singles.tile([P, 1], f32)
            nc.vector.memset(c_neg05[:], -0.5)

            # identity weights for TensorE c-sum matmuls: ident[p, j] = (j - p == 0)
            iota_t = singles.tile([P, P], mybir.dt.int32)
            nc.gpsimd.iota(iota_t[:], pattern=[[1, P]], base=0, channel_multiplier=-1)
            ident = singles.tile([P, P], bf16)
            nc.vector.tensor_scalar(out=ident[:], in0=iota_t[:], scalar1=0,
                                    scalar2=None, op0=Alu.is_equal)

            def csum_mm(psum_out, src_ap, n):
                """psum_out[p, j] = src[p, 3j] + src[p, 3j+1] + src[p, 3j+2], j < 2n.

                TensorE identity matmuls, accumulated in PSUM; groups of <=512
                output columns keep each matmul inside one PSUM bank.
                """
                ncols = 2 * n
                g0 = 0
                while g0 < ncols:
                    g1 = min(g0 + 512, ncols)
                    for cc in range(3):
                        rhs = bass.AP(tensor=src_ap.tensor,
                                      offset=src_ap.offset + 3 * g0 + cc,
                                      ap=[src_ap.ap[0], [3, g1 - g0]])
                        nc.tensor.matmul(out=psum_out[:, g0:g1],
                                         lhsT=ident[:], rhs=rhs,
                                         start=(cc == 0), stop=(cc == 2))
                    g0 = g1

            for c in range(C):
                t0 = c * W  # first owned timestep (within half)
                t_lo = max(0, t0 - 3)
                t_hi = min(HS, t0 + W + 3)
                col_lo = t_lo - (t0 - 3)
                ncols = t_hi - t_lo

                hand_t = inp.tile([P, E, 6], bf16)
                obj_t = inp.tile([P, E, 3], bf16)
                probs_t = inp.tile([P, E, 3], f32)

                loads = (
                    (hand_t, hand, 6, nc.sync),
                    (obj_t, obj, 3, nc.sync),
                    (probs_t, probs, 3, nc.scalar),
                )
                for tile_buf, ten, k, halo_eng in loads:
                    # Big main loads ride SWDGE.  Each dma_start instruction
                    # binds to ONE SDMA engine pair (round-robin over the 8
                    # pairs), so a chunk's inputs are split into 8 equal
                    # ~0.4MB instructions -- hand into 4, obj/probs into 2 --
                    # putting one load on every pair: the whole chunk streams
                    # in concurrently at ~8x a single pair's rate.
                    # chunk 0 loads at 2x granularity: its delivery latency
                    # is the head-of-kernel stall, halved by splitting finer.
                    nsub = (4 if k == 6 else 2) * (2 if c == 0 else 1)
                    rows = P // nsub
                    for s in range(nsub):
                        p0 = s * rows
                        b0 = p0 % H          # batch row within the half
                        half = p0 // H
                        nc.gpsimd.dma_start(
                            out=tile_buf[p0:p0 + rows, col_lo:col_lo + ncols, :],
                            in_=dram_ap(ten, (half * HS + t_lo) * k + b0 * seq * k,
                                        [[seq * k, rows], [1, ncols * k]]),
                        )
                    # halo loads: hand/obj tiles are bf16 (cast-in-DMA is
                    # SWDGE-only), so their halos ride gpsimd; probs (fp32)
                    # keeps its HWDGE halo.
                    h_eng = halo_eng if k == 3 and tile_buf is probs_t else nc.gpsimd
                    if c == 0:
                        h_eng.dma_start(
                            out=tile_buf[H:P, 0:3, :],
                            in_=dram_ap(ten, (HS - 3) * k,
                                        [[seq * k, bs_local], [1, 3 * k]]),
                        )
                        nc.vector.memset(tile_buf[0:H, 0:3, :], 0.0)
                    if c == C - 1:
                        h_eng.dma_start(
                            out=tile_buf[0:H, W + 3:E, :],
                            in_=dram_ap(ten, HS * k,
                                        [[seq * k, bs_local], [1, 3 * k]]),
                        )
                        nc.vector.memset(tile_buf[H:P, W + 3:E, :], 0.0)

                # ---- r = hand - obj (fp32 -> bf16) ----
                r_t = work.tile([P, E, 6], bf16)
                if R_BROADCAST:
                    ha = hand_t[:]
                    ra = r_t[:]
                    oa = obj_t[:]
                    hand_v = bass.AP(tensor=ha.tensor, offset=ha.offset,
                                     ap=[ha.ap[0], [6, E], [3, 2], [1, 3]])
                    r_v = bass.AP(tensor=ra.tensor, offset=ra.offset,
                                  ap=[ra.ap[0], [6, E], [3, 2], [1, 3]])
                    obj_v = bass.AP(tensor=oa.tensor, offset=oa.offset,
                                    ap=[oa.ap[0], [3, E], [0, 2], [1, 3]])
                    nc.vector.tensor_sub(r_v, hand_v, obj_v)
                else:
                    for h in range(2):
                        nc.vector.tensor_sub(r_t[:, :, 3 * h:3 * h + 3],
                                             hand_t[:, :, 3 * h:3 * h + 3], obj_t[:])

                # ---- d2 = sum_c r^2 (Square on ACT, c-sum on TensorE) ----
                sq_t = work.tile([P, E, 6], bf16)
                nc.scalar.activation(sq_t[:], r_t[:], Act.Square)

                # Input-only ops emitted early: they sit in the DVE queue
                # behind r and execute while ScalarE/TensorE run the
                # Square -> c-sum -> Sqrt chain (DVE queue is FIFO, so
                # chain-dependent ops emitted here would head-of-line block).
                cb_t = work.tile([P, W + 1, 2], bf16)
                nc.vector.tensor_scalar(
                    out=cb_t[:], in0=probs_t[:, 2:3 + W, 0:2],
                    scalar1=0.5, scalar2=None, op0=Alu.is_gt)
                # mask invalid t by zeroing cb columns (col k is t = t0+k-1):
                # fc[t] = (p[t-1] <= 0.5) & cb[t] reads cb at col t-t0+1.
                if c == 0:
                    nc.vector.memset(cb_t[0:H, 1:4, :], 0.0)  # t<3 (incl. forced-false t=0)
                if c == C - 1:
                    nc.vector.memset(cb_t[H:P, W - 2:W + 1, :], 0.0)  # t >= seq-3
                dr_t = work.tile([P, W, 6], bf16)
                nc.vector.tensor_sub(dr_t[:], r_t[:, 4:4 + W, :], r_t[:, 3:3 + W, :])
                fc_t = work.tile([P, W, 2], bf16)
                nc.vector.scalar_tensor_tensor(
                    out=fc_t[:], in0=probs_t[:, 2:2 + W, 0:2], scalar=0.5,
                    in1=cb_t[:, 1:W + 1, :],
                    op0=Alu.is_le, op1=Alu.logical_and, accum_out=cns[:, c:c + 1])
                d2_ps = psum.tile([P, 2 * E], f32)
                csum_mm(d2_ps, sq_t[:].opt(), E)
                d_t = work.tile([P, E, 2], bf16)
                # sqrt-d ahead of dsq in the ScalarE FIFO: the smoothness
                # chain (e/sdp/...) waits on d, while the velocity chain has
                # slack until l2 at the chunk tail.
                nc.scalar.activation(d_t[:].opt(), d2_ps[:], Act.Sqrt)
                dsq_t = work.tile([P, W, 6], bf16)
                nc.scalar.activation(dsq_t[:], dr_t[:], Act.Square)
                v2_ps = psum.tile([P, 2 * W], f32)
                csum_mm(v2_ps, dsq_t[:].opt(), W)

                # ---- smoothness diffs (depend on d) ----
                e_t = work.tile([P, E - 1, 2], bf16)
                nc.vector.tensor_sub(e_t[:], d_t[:, 1:E, :], d_t[:, 0:E - 1, :])
                sdp_t = work.tile([P, W + 4, 2], bf16)
                nc.vector.tensor_sub(sdp_t[:], e_t[:, 0:W + 4, :], e_t[:, 1:W + 5, :])
                sd_t = work.tile([P, W + 4, 2], bf16)
                nc.scalar.activation(sd_t[:], sdp_t[:], Act.Abs)
                s2_t = work.tile([P, W + 3, 2], bf16)
                nc.vector.tensor_add(s2_t[:], sd_t[:, 0:W + 3, :], sd_t[:, 1:W + 4, :])
                s4_t = work.tile([P, W + 1, 2], bf16)
                nc.vector.tensor_add(s4_t[:], s2_t[:, 0:W + 1, :], s2_t[:, 2:W + 3, :])
                sm5_t = work.tile([P, W, 2], bf16)
                nc.vector.tensor_add(sm5_t[:], s4_t[:, 0:W, :], sd_t[:, 4:W + 4, :])

                # ---- contact distance partial ----
                derr_t = work.tile([P, W, 2], bf16)
                nc.scalar.activation(derr_t[:], d_t[:, 3:3 + W, :], Act.Abs, bias=c_neg01[:])
                l1p_t = work.tile([P, W, 2], bf16)
                nc.vector.scalar_tensor_tensor(
                    out=l1p_t[:], in0=probs_t[:, 3:3 + W, 0:2], scalar=1.0, in1=derr_t[:],
                    op0=Alu.mult, op1=Alu.mult, accum_out=l1s[:, c:c + 1])

                # ---- velocity partial ----
                vd_t = work.tile([P, W, 2], bf16)
                nc.scalar.activation(vd_t[:].opt(), v2_ps[:], Act.Sqrt)
                if c == C - 1:
                    nc.vector.memset(vd_t[H:P, W - 1:W, :], 0.0)  # j=seq-1 invalid
                l2p_t = work.tile([P, W, 2], bf16)
                nc.vector.scalar_tensor_tensor(
                    out=l2p_t[:], in0=probs_t[:, 4:4 + W, 0:2], scalar=1.0, in1=vd_t[:],
                    op0=Alu.mult, op1=Alu.mult, accum_out=l2s[:, c:c + 1])

                smp_t = work.tile([P, W, 2], bf16)
                nc.vector.scalar_tensor_tensor(
                    out=smp_t[:], in0=sm5_t[:], scalar=1.0, in1=fc_t[:],
                    op0=Alu.mult, op1=Alu.mult, accum_out=sms[:, c:c + 1])

            # ---- final per-partition combine + store ----
            for i, slot in enumerate((l1s, l2s, sms, cns)):
                nc.vector.tensor_reduce(outt[:, i:i + 1], slot[:], axis=mybir.AxisListType.X, op=Alu.add)
            nc.sync.dma_start(out=partials.ap(), in_=outt[:])

    nc.compile()
    return nc


_cache = {}


def _get_nc(bs_local, seq, W):
    key = (bs_local, seq, W)
    if key not in _cache:
        _cache[key] = build_nc(bs_local, seq, W)
    return _cache[key]


def combine_partials(parts, bs, seq, training_step):
    """parts: float array [..., 4] of per-core/per-partition partial sums."""
    s = np.asarray(parts, dtype=np.float64).reshape(-1, 4).sum(axis=0)
    l1 = s[0] / (bs * seq * 2)
    l2 = s[1] / (bs * (seq - 1) * 2) if seq > 1 else 0.0
    cnt = s[3]
    sm = (s[2] / 5.0) / max(cnt, 1.0) if (seq > 5 and cnt > 0) else 0.0
    ramp = min(1.0, float(training_step) / 1000.0)
    return np.array(ramp * (1.0 * l1 + 0.5 * l2 + 0.3 * sm), dtype=np.float32)


def _run(pred_hand_pos, pred_obj_pos, contact_probs, **spmd_kwargs):
    from concourse.bass_utils import run_bass_kernel_spmd

    hand = np.ascontiguousarray(np.asarray(pred_hand_pos, dtype=np.float32))
    obj = np.ascontiguousarray(np.asarray(pred_obj_pos, dtype=np.float32))
    probs = np.ascontiguousarray(np.asarray(contact_probs, dtype=np.float32))
    bs, seq = hand.shape[:2]
    bs_local = bs // N_CORES
    nc = _get_nc(bs_local, seq, W_FULL)

    in_maps = []
    for i in range(N_CORES):
        sl = slice(i * bs_local, (i + 1) * bs_local)
        in_maps.append({
            "pred_hand_pos": hand[sl],
            "pred_obj_pos": obj[sl],
            "contact_probs": probs[sl],
        })
    # The axon terminal occasionally reports the exec unit unrecoverable on
    # the first touch after a previous process's teardown; a retry lands on a
    # recovered device.
    last_err = None
    for _ in range(3):
        try:
            res = run_bass_kernel_spmd(
                nc, in_maps, core_ids=list(range(N_CORES)), **spmd_kwargs
            )
            parts = np.stack([res.results[i]["partials"] for i in range(N_CORES)])
            return parts, res
        except Exception as e:  # noqa: BLE001
            last_err = e
    raise last_err


def kernel(pred_hand_pos, pred_obj_pos, contact_probs, training_step):
    bs, seq = np.asarray(pred_hand_pos).shape[:2]
    parts, _ = _run(pred_hand_pos, pred_obj_pos, contact_probs)
    return combine_partials(parts, bs, seq, training_step)
